# revision 9
# baseline (speedup 1.0000x reference)
"""Trainium2 Bass kernel for nn_Attention_65094524338925.

Reference computation (B=64, S=2048, H=256):
    m_text   = text  @ W_text.T          [B,S,H]
    m_aspect = aspect @ W_text.T         [B,S,H]
    combine  = tanh([m_text, m_aspect])  [B,S,2H]
    score    = combine @ W_combine.T     [B,S,1]
    weight   = softmax(score, axis=1)    -> transposed to [B,1,S]
    out      = weight @ text             [B,1,H]
    returns (weight, out)

Strategy: data-parallel over batch, 8 examples per NeuronCore. The host
shards and lays out inputs so the device never needs an on-chip transpose
of the big activations:
  - xt:  concat(text.T, aspect.T) per example, bf16  [hc, 128, 4096]
         (PE contracts over the partition dim, so h lives on partitions)
  - xn:  text in natural [s, h] layout, bf16 (for the final weight @ text)
  - wt:  W_text.T chunks (stationary operand of the first matmul)
  - wcx: W_combine halves as one-hot-column [128, 8] stationary tiles
         (matmul PSUM output must start at partition 0/32/64, so each
         example's score row is produced by an M=8 matmul whose lhsT has
         only column b nonzero)

Per example: PE computes m.T [k, s] in PSUM, ACT applies tanh (psum->sbuf,
bf16), PE dots tanh rows with W_combine into a shared [8, 2048] score
accumulator. Tail: one batched exp with fused row-sum (softmax without max
subtraction -- scores are O(0.1) so exp cannot overflow), normalize on DVE,
DMA-transpose the weights to [s, b] layout, and PE reduces weight @ text.
"""

import numpy as np
import ml_dtypes

import concourse.bass as bass
import concourse.mybir as mybir
import concourse.tile as tile
from concourse.bass_utils import run_bass_kernel_spmd

BF16 = ml_dtypes.bfloat16
F32 = mybir.dt.float32
BF = mybir.dt.bfloat16

N_CORES = 8
B, S, H = 64, 2048, 256
BL = B // N_CORES  # 8 examples per core
HC = H // 128  # 2 h-chunks
SCAT = 2 * S  # concat(text, aspect) along s: 4096


# ---------------------------------------------------------------------------
# Workaround: this walrus build accepts at most one sync wait per TPB
# instruction (64B instruction formats carry a single wait slot). Tile's
# scheduler attaches one wait per outstanding proc, so (1) spill excess
# waits onto preceding same-engine NOPs and (2) replace the exit drain's
# multi-proc wait list with per-proc sync NOPs.
# ---------------------------------------------------------------------------
_PATCHED = False


def _apply_tile_patches():
    global _PATCHED
    if _PATCHED:
        return
    _PATCHED = True
    from concourse.vector_clock import ScopedClock, VectorClock

    max_waits = 1
    ctr = [0]
    orig_lower = tile.TileContext._lower_ordered_insts

    def split_waits(ordered):
        for bb_name, insts in ordered.items():
            new = []
            for inst in insts:
                si = inst.sync_info
                if si is not None and si.on_wait and len(si.on_wait) > max_waits:
                    waits = list(si.on_wait)
                    spill, keep = waits[:-max_waits], waits[-max_waits:]
                    for w in spill:
                        ctr[0] += 1
                        new.append(
                            mybir.InstNoOp(
                                name=f"waitspill_{ctr[0]}",
                                engine=inst.engine,
                                sync_info=mybir.SyncInfo(on_wait=[w], on_update=[]),
                                bass_nofuse=True,
                            )
                        )
                    inst.sync_info = mybir.SyncInfo(
                        on_wait=keep, on_update=list(si.on_update or [])
                    )
                new.append(inst)
            ordered[bb_name] = new

    def patched_lower(self, ordered):
        split_waits(ordered)
        return orig_lower(self, ordered)

    def patched_drain_and_barrier(self, tick_clock, wait_clock):
        gc = tick_clock.global_clock
        n = len(gc)
        for p in range(n):
            t = gc[p]
            if t <= 0:
                continue
            vec = [0] * n
            vec[p] = t
            nop = self.nc.sync.nop()
            wait_clock.add_sem_waits(nop.ins, ScopedClock({None: VectorClock(vec)}))
        self.nc.sync.drain()
        self.nc.all_engine_barrier()
        assert self.sems is not None
        popped = self.nc._tile_sem_poison_stack.pop()
        assert popped is self._sem_poison
        self.nc.clear_and_free_semaphores(list(self.sems.allocated().values()))
        self.nc.all_engine_barrier()

    tile.TileContext._lower_ordered_insts = patched_lower
    tile.TileContext._drain_and_barrier = patched_drain_and_barrier


# ---------------------------------------------------------------------------
# Kernel build
# ---------------------------------------------------------------------------
_NC_CACHE = None


def build_nc():
    global _NC_CACHE
    if _NC_CACHE is not None:
        return _NC_CACHE
    _apply_tile_patches()

    nc = bass.Bass("TRN2", num_devices=N_CORES, debug=False)
    xt = nc.declare_dram_parameter("xt", [BL, HC, 128, SCAT], mybir.dt.bfloat16, isOutput=False)
    xn = nc.declare_dram_parameter("xn", [BL, S, H], mybir.dt.bfloat16, isOutput=False)
    wt = nc.declare_dram_parameter("wt", [HC, 128, H], mybir.dt.bfloat16, isOutput=False)
    wcx = nc.declare_dram_parameter("wcx", [2, 2, BL, 128, BL], mybir.dt.bfloat16, isOutput=False)
    weight = nc.declare_dram_parameter("weight", [BL, S], F32, isOutput=True)
    out = nc.declare_dram_parameter("out", [BL, H], F32, isOutput=True)

    Tanh = mybir.ActivationFunctionType.Tanh
    Exp = mybir.ActivationFunctionType.Exp
    Copy = mybir.ActivationFunctionType.Copy

    with tile.TileContext(nc) as tc:
        with (
            tc.tile_pool(name="consts", bufs=1) as consts,
            tc.tile_pool(name="xtp", bufs=4) as xtp,
            tc.tile_pool(name="xnp", bufs=8) as xnp,
            tc.tile_pool(name="thp", bufs=4) as thp,
            tc.tile_pool(name="tailp", bufs=1) as tailp,
            tc.tile_pool(name="outp", bufs=2) as outp,
            tc.tile_pool(name="mps", bufs=3, space="PSUM") as mps,
            tc.tile_pool(name="sps", bufs=2, space="PSUM") as sps,
        ):
            # constants
            wt_sb = consts.tile([128, HC, H], BF)
            nc.sync.dma_start(out=wt_sb[:], in_=wt.rearrange("hc p k -> p hc k"))
            wcx_sb = consts.tile([128, 2, 2, BL, BL], BF)
            nc.sync.dma_start(out=wcx_sb[:], in_=wcx.rearrange("kc j b2 p b -> p kc j b2 b"))

            # score accumulator lives in SBUF; each half's contribution is
            # produced in a small PSUM scratch and added in on DVE
            score_sb = tailp.tile([BL, S], F32)
            nc.vector.memset(score_sb[:], 0.0)

            xn_sb = {}
            for b in range(BL):
                xt_sb = [xtp.tile([128, SCAT], BF, tag="xt", name=f"xt_sb_{b}_{hc}") for hc in range(HC)]
                for half in range(2):
                    for hc in range(HC):
                        nc.sync.dma_start(
                            out=xt_sb[hc][:, half * S : (half + 1) * S],
                            in_=xt[b, hc, :, half * S : (half + 1) * S],
                        )
                xn_sb[b] = xnp.tile([128, S // 128, H], BF, tag="xn", name=f"xn_sb_{b}")
                nc.gpsimd.dma_start(
                    out=xn_sb[b][:], in_=xn[b].rearrange("(t p) h -> p t h", p=128)
                )

                for sh in range(4):  # four 1024-col halves of the concat stream
                    j = 0 if sh < 2 else 1  # w1 for text halves, w2 for aspect halves
                    col0 = (sh % 2) * 1024  # output score columns
                    th = []
                    for kc in range(2):
                        mt = mps.tile([128, 1024], F32, tag="m")
                        for hc in range(HC):
                            lhsT = wt_sb[:, hc, kc * 128 : (kc + 1) * 128]
                            for sc in range(2):
                                nc.tensor.matmul(
                                    out=mt[:, sc * 512 : (sc + 1) * 512],
                                    lhsT=lhsT,
                                    rhs=xt_sb[hc][
                                        :, sh * 1024 + sc * 512 : sh * 1024 + (sc + 1) * 512
                                    ],
                                    start=(hc == 0),
                                    stop=(hc == HC - 1),
                                )
                        tht = thp.tile([128, 1024], BF, tag="th")
                        nc.scalar.activation(out=tht[:], in_=mt[:], func=Tanh)
                        th.append(tht)
                    for sc in range(2):
                        ssc = sps.tile([BL, 512], F32, tag="ssc", name=f"ssc_{b}_{sh}_{sc}")
                        for kc in range(2):
                            nc.tensor.matmul(
                                out=ssc[:],
                                lhsT=wcx_sb[:, kc, j, b, :],
                                rhs=th[kc][:, sc * 512 : (sc + 1) * 512],
                                start=(kc == 0),
                                stop=(kc == 1),
                            )
                        c0 = col0 + sc * 512
                        nc.vector.tensor_add(
                            score_sb[:, c0 : c0 + 512],
                            score_sb[:, c0 : c0 + 512],
                            ssc[:],
                        )

            # ---- softmax tail (batched across the 8 examples) ----
            e_f32 = tailp.tile([BL, S], F32)
            z_sb = tailp.tile([BL, 1], F32)
            nc.scalar.activation(
                out=e_f32[:], in_=score_sb[:], func=Exp, accum_out=z_sb[:]
            )
            rz = tailp.tile([BL, 1], F32)
            nc.vector.reciprocal(rz[:], z_sb[:])
            nc.vector.tensor_scalar_mul(e_f32[:], e_f32[:], rz[:])
            nc.sync.dma_start(out=weight[:], in_=e_f32[:])

            # normalized weights in bf16, padded to 16 partitions for the
            # DMA xbar transpose (partition count must be a multiple of 16)
            e_bf = tailp.tile([16, S], BF)
            nc.vector.memset(e_bf[:], 0.0)
            nc.vector.tensor_copy(e_bf[0:BL, :], e_f32[:])
            et = tailp.tile([128, S // 128, 16], BF)
            nc.sync.dma_start_transpose(out=et[:], in_=e_bf[:])

            # out[b] = sum_s weight[b, s] * text[b, s, :]
            for b in range(BL):
                ops = sps.tile([1, H], F32, tag="ssc", name=f"ops_{b}")
                for c in range(S // 128):
                    nc.tensor.matmul(
                        out=ops[:],
                        lhsT=et[:, c, b : b + 1],
                        rhs=xn_sb[b][:, c, :],
                        start=(c == 0),
                        stop=(c == S // 128 - 1),
                    )
                orow = outp.tile([1, H], F32, tag="orow")
                nc.scalar.activation(out=orow[:], in_=ops[:], func=Copy)
                nc.sync.dma_start(out=out[b : b + 1, :], in_=orow[:])

    _NC_CACHE = nc
    return nc


# ---------------------------------------------------------------------------
# Host-side sharding / layout
# ---------------------------------------------------------------------------
def _prep_in_maps(text, aspect, W_text, W_combine):
    text = np.asarray(text, dtype=np.float32)
    aspect = np.asarray(aspect, dtype=np.float32)
    W_text = np.asarray(W_text, dtype=np.float32)
    W_combine = np.asarray(W_combine, dtype=np.float32)

    wt_np = np.ascontiguousarray(W_text.T.reshape(HC, 128, H)).astype(BF16)
    wcx_np = np.zeros((2, 2, BL, 128, BL), dtype=BF16)
    for kc in range(2):
        for j in range(2):
            seg = W_combine[0, j * H + kc * 128 : j * H + (kc + 1) * 128].astype(BF16)
            for b in range(BL):
                wcx_np[kc, j, b, :, b] = seg

    in_maps = []
    for i in range(N_CORES):
        tb = text[i * BL : (i + 1) * BL]  # [8, S, H]
        ab = aspect[i * BL : (i + 1) * BL]
        cat = np.concatenate([tb.transpose(0, 2, 1), ab.transpose(0, 2, 1)], axis=2)
        xt_np = np.ascontiguousarray(cat).astype(BF16).reshape(BL, HC, 128, SCAT)
        xn_np = tb.astype(BF16)
        in_maps.append(
            {"xt": xt_np, "xn": xn_np, "wt": wt_np, "wcx": wcx_np}
        )
    return in_maps


def _run(inputs, trace=False, tmpdir=None):
    nc = build_nc()
    in_maps = _prep_in_maps(
        inputs["text"], inputs["aspect"], inputs["W_text"], inputs["W_combine"]
    )
    res = run_bass_kernel_spmd(
        nc, in_maps, list(range(N_CORES)), trace=trace, tmpdir=tmpdir
    )
    weight = np.stack([res.results[i]["weight"] for i in range(N_CORES)]).reshape(
        B, 1, S
    )
    out = np.stack([res.results[i]["out"] for i in range(N_CORES)]).reshape(B, 1, H)
    return (weight.astype(np.float32), out.astype(np.float32)), res


def kernel(**inputs):
    outputs, _ = _run(inputs, trace=False)
    return outputs


# revision 10
# speedup vs baseline: 1.1119x; 1.1119x over previous
"""Trainium2 Bass kernel for nn_Attention_65094524338925.

Reference computation (B=64, S=2048, H=256):
    m_text   = text  @ W_text.T          [B,S,H]
    m_aspect = aspect @ W_text.T         [B,S,H]
    combine  = tanh([m_text, m_aspect])  [B,S,2H]
    score    = combine @ W_combine.T     [B,S,1]
    weight   = softmax(score, axis=1)    -> transposed to [B,1,S]
    out      = weight @ text             [B,1,H]
    returns (weight, out)

Strategy: data-parallel over batch, 8 examples per NeuronCore. The host
shards and lays out inputs so the device never needs an on-chip transpose
of the big activations:
  - xt:  concat(text.T, aspect.T) per example, bf16  [hc, 128, 4096]
         (PE contracts over the partition dim, so h lives on partitions)
  - xn:  text in natural [s, h] layout, bf16 (for the final weight @ text)
  - wt:  W_text.T chunks (stationary operand of the first matmul)
  - wcx: W_combine halves as one-hot-column [128, 8] stationary tiles
         (matmul PSUM output must start at partition 0/32/64, so each
         example's score row is produced by an M=8 matmul whose lhsT has
         only column b nonzero)

Per example: PE computes m.T [k, s] in PSUM, ACT applies tanh (psum->sbuf,
bf16), PE dots tanh rows with W_combine into a shared [8, 2048] score
accumulator. Tail: one batched exp with fused row-sum (softmax without max
subtraction -- scores are O(0.1) so exp cannot overflow), normalize on DVE,
DMA-transpose the weights to [s, b] layout, and PE reduces weight @ text.
"""

import numpy as np
import ml_dtypes

import concourse.bass as bass
import concourse.mybir as mybir
import concourse.tile as tile
from concourse.bass_utils import run_bass_kernel_spmd

BF16 = ml_dtypes.bfloat16
F32 = mybir.dt.float32
BF = mybir.dt.bfloat16

N_CORES = 8
B, S, H = 64, 2048, 256
BL = B // N_CORES  # 8 examples per core
HC = H // 128  # 2 h-chunks
SCAT = 2 * S  # concat(text, aspect) along s: 4096


# ---------------------------------------------------------------------------
# Workaround: this walrus build accepts at most one sync wait per TPB
# instruction (64B instruction formats carry a single wait slot). Tile's
# scheduler attaches one wait per outstanding proc, so (1) spill excess
# waits onto preceding same-engine NOPs and (2) replace the exit drain's
# multi-proc wait list with per-proc sync NOPs.
# ---------------------------------------------------------------------------
_PATCHED = False


def _apply_tile_patches():
    global _PATCHED
    if _PATCHED:
        return
    _PATCHED = True
    from concourse.vector_clock import ScopedClock, VectorClock

    max_waits = 1
    ctr = [0]
    orig_lower = tile.TileContext._lower_ordered_insts

    def split_waits(ordered):
        for bb_name, insts in ordered.items():
            new = []
            for inst in insts:
                si = inst.sync_info
                if si is not None and si.on_wait and len(si.on_wait) > max_waits:
                    waits = list(si.on_wait)
                    spill, keep = waits[:-max_waits], waits[-max_waits:]
                    for w in spill:
                        ctr[0] += 1
                        new.append(
                            mybir.InstNoOp(
                                name=f"waitspill_{ctr[0]}",
                                engine=inst.engine,
                                sync_info=mybir.SyncInfo(on_wait=[w], on_update=[]),
                                bass_nofuse=True,
                            )
                        )
                    inst.sync_info = mybir.SyncInfo(
                        on_wait=keep, on_update=list(si.on_update or [])
                    )
                new.append(inst)
            ordered[bb_name] = new

    def patched_lower(self, ordered):
        split_waits(ordered)
        return orig_lower(self, ordered)

    def patched_drain_and_barrier(self, tick_clock, wait_clock):
        gc = tick_clock.global_clock
        n = len(gc)
        for p in range(n):
            t = gc[p]
            if t <= 0:
                continue
            vec = [0] * n
            vec[p] = t
            nop = self.nc.sync.nop()
            wait_clock.add_sem_waits(nop.ins, ScopedClock({None: VectorClock(vec)}))
        self.nc.sync.drain()
        self.nc.all_engine_barrier()
        assert self.sems is not None
        popped = self.nc._tile_sem_poison_stack.pop()
        assert popped is self._sem_poison
        self.nc.clear_and_free_semaphores(list(self.sems.allocated().values()))
        self.nc.all_engine_barrier()

    tile.TileContext._lower_ordered_insts = patched_lower
    tile.TileContext._drain_and_barrier = patched_drain_and_barrier


# ---------------------------------------------------------------------------
# Kernel build
# ---------------------------------------------------------------------------
_NC_CACHE = None


def build_nc():
    global _NC_CACHE
    if _NC_CACHE is not None:
        return _NC_CACHE
    _apply_tile_patches()

    nc = bass.Bass("TRN2", num_devices=N_CORES, debug=False)
    xt = nc.declare_dram_parameter("xt", [BL, HC, 128, SCAT], mybir.dt.bfloat16, isOutput=False)
    xn = nc.declare_dram_parameter("xn", [BL, S, H], mybir.dt.bfloat16, isOutput=False)
    wt = nc.declare_dram_parameter("wt", [HC, 128, H], mybir.dt.bfloat16, isOutput=False)
    wcx = nc.declare_dram_parameter("wcx", [2, 2, BL, 128, BL], mybir.dt.bfloat16, isOutput=False)
    weight = nc.declare_dram_parameter("weight", [BL, S], F32, isOutput=True)
    out = nc.declare_dram_parameter("out", [BL, H], F32, isOutput=True)

    Tanh = mybir.ActivationFunctionType.Tanh
    Exp = mybir.ActivationFunctionType.Exp
    Copy = mybir.ActivationFunctionType.Copy

    with tile.TileContext(nc) as tc:
        with (
            tc.tile_pool(name="consts", bufs=1) as consts,
            tc.tile_pool(name="xtp", bufs=4) as xtp,
            tc.tile_pool(name="xnp", bufs=8) as xnp,
            tc.tile_pool(name="thp", bufs=4) as thp,
            tc.tile_pool(name="tailp", bufs=1) as tailp,
            tc.tile_pool(name="outp", bufs=2) as outp,
            tc.tile_pool(name="mps", bufs=3, space="PSUM") as mps,
            tc.tile_pool(name="sps", bufs=2, space="PSUM") as sps,
        ):
            # constants
            wt_sb = consts.tile([128, HC, H], BF)
            nc.sync.dma_start(out=wt_sb[:], in_=wt.rearrange("hc p k -> p hc k"))
            wcx_sb = consts.tile([128, 2, 2, BL, BL], BF)
            nc.sync.dma_start(out=wcx_sb[:], in_=wcx.rearrange("kc j b2 p b -> p kc j b2 b"))

            # score accumulator lives in SBUF; each half's contribution is
            # produced in a small PSUM scratch and added in on DVE
            score_sb = tailp.tile([BL, S], F32)
            nc.vector.memset(score_sb[:], 0.0)

            xn_sb = {}
            for b in range(BL):
                xt_sb = [xtp.tile([128, SCAT], BF, tag="xt", name=f"xt_sb_{b}_{hc}") for hc in range(HC)]
                for half in range(2):
                    for hc in range(HC):
                        nc.sync.dma_start(
                            out=xt_sb[hc][:, half * S : (half + 1) * S],
                            in_=xt[b, hc, :, half * S : (half + 1) * S],
                        )
                for sh in range(4):  # four 1024-col halves of the concat stream
                    j = 0 if sh < 2 else 1  # w1 for text halves, w2 for aspect halves
                    col0 = (sh % 2) * 1024  # output score columns
                    th = []
                    for kc in range(2):
                        mt = mps.tile([128, 1024], F32, tag="m")
                        for hc in range(HC):
                            lhsT = wt_sb[:, hc, kc * 128 : (kc + 1) * 128]
                            for sc in range(2):
                                nc.tensor.matmul(
                                    out=mt[:, sc * 512 : (sc + 1) * 512],
                                    lhsT=lhsT,
                                    rhs=xt_sb[hc][
                                        :, sh * 1024 + sc * 512 : sh * 1024 + (sc + 1) * 512
                                    ],
                                    start=(hc == 0),
                                    stop=(hc == HC - 1),
                                )
                        tht = thp.tile([128, 1024], BF, tag="th")
                        nc.scalar.activation(out=tht[:], in_=mt[:], func=Tanh)
                        th.append(tht)
                    for sc in range(2):
                        ssc = sps.tile([BL, 512], F32, tag="ssc", name=f"ssc_{b}_{sh}_{sc}")
                        for kc in range(2):
                            nc.tensor.matmul(
                                out=ssc[:],
                                lhsT=wcx_sb[:, kc, j, b, :],
                                rhs=th[kc][:, sc * 512 : (sc + 1) * 512],
                                start=(kc == 0),
                                stop=(kc == 1),
                            )
                        c0 = col0 + sc * 512
                        nc.vector.tensor_add(
                            score_sb[:, c0 : c0 + 512],
                            score_sb[:, c0 : c0 + 512],
                            ssc[:],
                        )

                xn_sb[b] = xnp.tile([128, S // 128, H], BF, tag="xn", name=f"xn_sb_{b}")
                nc.gpsimd.dma_start(
                    out=xn_sb[b][:], in_=xn[b].rearrange("(t p) h -> p t h", p=128)
                )

            # ---- softmax tail (batched across the 8 examples) ----
            e_f32 = tailp.tile([BL, S], F32)
            z_sb = tailp.tile([BL, 1], F32)
            nc.scalar.activation(
                out=e_f32[:], in_=score_sb[:], func=Exp, accum_out=z_sb[:]
            )
            rz = tailp.tile([BL, 1], F32)
            nc.vector.reciprocal(rz[:], z_sb[:])
            nc.vector.tensor_scalar_mul(e_f32[:], e_f32[:], rz[:])
            nc.sync.dma_start(out=weight[:], in_=e_f32[:])

            # normalized weights in bf16, padded to 16 partitions for the
            # DMA xbar transpose (partition count must be a multiple of 16)
            e_bf = tailp.tile([16, S], BF)
            nc.vector.memset(e_bf[:], 0.0)
            nc.vector.tensor_copy(e_bf[0:BL, :], e_f32[:])
            et = tailp.tile([128, S // 128, 16], BF)
            nc.sync.dma_start_transpose(out=et[:], in_=e_bf[:])

            # out[b] = sum_s weight[b, s] * text[b, s, :]
            for b in range(BL):
                ops = sps.tile([1, H], F32, tag="ssc", name=f"ops_{b}")
                for c in range(S // 128):
                    nc.tensor.matmul(
                        out=ops[:],
                        lhsT=et[:, c, b : b + 1],
                        rhs=xn_sb[b][:, c, :],
                        start=(c == 0),
                        stop=(c == S // 128 - 1),
                    )
                orow = outp.tile([1, H], F32, tag="orow")
                nc.scalar.activation(out=orow[:], in_=ops[:], func=Copy)
                nc.sync.dma_start(out=out[b : b + 1, :], in_=orow[:])

    _NC_CACHE = nc
    return nc


# ---------------------------------------------------------------------------
# Host-side sharding / layout
# ---------------------------------------------------------------------------
def _prep_in_maps(text, aspect, W_text, W_combine):
    text = np.asarray(text, dtype=np.float32)
    aspect = np.asarray(aspect, dtype=np.float32)
    W_text = np.asarray(W_text, dtype=np.float32)
    W_combine = np.asarray(W_combine, dtype=np.float32)

    wt_np = np.ascontiguousarray(W_text.T.reshape(HC, 128, H)).astype(BF16)
    wcx_np = np.zeros((2, 2, BL, 128, BL), dtype=BF16)
    for kc in range(2):
        for j in range(2):
            seg = W_combine[0, j * H + kc * 128 : j * H + (kc + 1) * 128].astype(BF16)
            for b in range(BL):
                wcx_np[kc, j, b, :, b] = seg

    in_maps = []
    for i in range(N_CORES):
        tb = text[i * BL : (i + 1) * BL]  # [8, S, H]
        ab = aspect[i * BL : (i + 1) * BL]
        cat = np.concatenate([tb.transpose(0, 2, 1), ab.transpose(0, 2, 1)], axis=2)
        xt_np = np.ascontiguousarray(cat).astype(BF16).reshape(BL, HC, 128, SCAT)
        xn_np = tb.astype(BF16)
        in_maps.append(
            {"xt": xt_np, "xn": xn_np, "wt": wt_np, "wcx": wcx_np}
        )
    return in_maps


def _run(inputs, trace=False, tmpdir=None):
    nc = build_nc()
    in_maps = _prep_in_maps(
        inputs["text"], inputs["aspect"], inputs["W_text"], inputs["W_combine"]
    )
    res = run_bass_kernel_spmd(
        nc, in_maps, list(range(N_CORES)), trace=trace, tmpdir=tmpdir
    )
    weight = np.stack([res.results[i]["weight"] for i in range(N_CORES)]).reshape(
        B, 1, S
    )
    out = np.stack([res.results[i]["out"] for i in range(N_CORES)]).reshape(B, 1, H)
    return (weight.astype(np.float32), out.astype(np.float32)), res


def kernel(**inputs):
    outputs, _ = _run(inputs, trace=False)
    return outputs


# revision 11
# speedup vs baseline: 1.2315x; 1.1076x over previous
"""Trainium2 Bass kernel for nn_Attention_65094524338925.

Reference computation (B=64, S=2048, H=256):
    m_text   = text  @ W_text.T          [B,S,H]
    m_aspect = aspect @ W_text.T         [B,S,H]
    combine  = tanh([m_text, m_aspect])  [B,S,2H]
    score    = combine @ W_combine.T     [B,S,1]
    weight   = softmax(score, axis=1)    -> transposed to [B,1,S]
    out      = weight @ text             [B,1,H]
    returns (weight, out)

Strategy: data-parallel over batch, 8 examples per NeuronCore. The host
shards and lays out inputs so the device never needs an on-chip transpose
of the big activations:
  - xt:  concat(text.T, aspect.T) per example, bf16  [hc, 128, 4096]
         (PE contracts over the partition dim, so h lives on partitions)
  - xn:  text in natural [s, h] layout, bf16 (for the final weight @ text)
  - wt:  W_text.T chunks (stationary operand of the first matmul)
  - wcx: W_combine halves as one-hot-column [128, 8] stationary tiles
         (matmul PSUM output must start at partition 0/32/64, so each
         example's score row is produced by an M=8 matmul whose lhsT has
         only column b nonzero)

Per example: PE computes m.T [k, s] in PSUM, ACT applies tanh (psum->sbuf,
bf16), PE dots tanh rows with W_combine into a shared [8, 2048] score
accumulator. Tail: one batched exp with fused row-sum (softmax without max
subtraction -- scores are O(0.1) so exp cannot overflow), normalize on DVE,
DMA-transpose the weights to [s, b] layout, and PE reduces weight @ text.
"""

import numpy as np
import ml_dtypes

import concourse.bass as bass
import concourse.mybir as mybir
import concourse.tile as tile
from concourse.bass_utils import run_bass_kernel_spmd

BF16 = ml_dtypes.bfloat16
F32 = mybir.dt.float32
BF = mybir.dt.bfloat16

N_CORES = 8
B, S, H = 64, 2048, 256
BL = B // N_CORES  # 8 examples per core
HC = H // 128  # 2 h-chunks
SCAT = 2 * S  # concat(text, aspect) along s: 4096


# ---------------------------------------------------------------------------
# Workaround: this walrus build accepts at most one sync wait per TPB
# instruction (64B instruction formats carry a single wait slot). Tile's
# scheduler attaches one wait per outstanding proc, so (1) spill excess
# waits onto preceding same-engine NOPs and (2) replace the exit drain's
# multi-proc wait list with per-proc sync NOPs.
# ---------------------------------------------------------------------------
_PATCHED = False


def _apply_tile_patches():
    global _PATCHED
    if _PATCHED:
        return
    _PATCHED = True
    from concourse.vector_clock import ScopedClock, VectorClock

    max_waits = 1
    ctr = [0]
    orig_lower = tile.TileContext._lower_ordered_insts

    def split_waits(ordered):
        for bb_name, insts in ordered.items():
            new = []
            for inst in insts:
                si = inst.sync_info
                if si is not None and si.on_wait and len(si.on_wait) > max_waits:
                    waits = list(si.on_wait)
                    spill, keep = waits[:-max_waits], waits[-max_waits:]
                    for w in spill:
                        ctr[0] += 1
                        new.append(
                            mybir.InstNoOp(
                                name=f"waitspill_{ctr[0]}",
                                engine=inst.engine,
                                sync_info=mybir.SyncInfo(on_wait=[w], on_update=[]),
                                bass_nofuse=True,
                            )
                        )
                    inst.sync_info = mybir.SyncInfo(
                        on_wait=keep, on_update=list(si.on_update or [])
                    )
                new.append(inst)
            ordered[bb_name] = new

    def patched_lower(self, ordered):
        split_waits(ordered)
        return orig_lower(self, ordered)

    def patched_drain_and_barrier(self, tick_clock, wait_clock):
        gc = tick_clock.global_clock
        n = len(gc)
        for p in range(n):
            t = gc[p]
            if t <= 0:
                continue
            vec = [0] * n
            vec[p] = t
            nop = self.nc.sync.nop()
            wait_clock.add_sem_waits(nop.ins, ScopedClock({None: VectorClock(vec)}))
        self.nc.sync.drain()
        self.nc.all_engine_barrier()
        assert self.sems is not None
        popped = self.nc._tile_sem_poison_stack.pop()
        assert popped is self._sem_poison
        self.nc.clear_and_free_semaphores(list(self.sems.allocated().values()))
        self.nc.all_engine_barrier()

    tile.TileContext._lower_ordered_insts = patched_lower
    tile.TileContext._drain_and_barrier = patched_drain_and_barrier


# ---------------------------------------------------------------------------
# Kernel build
# ---------------------------------------------------------------------------
_NC_CACHE = None


def build_nc():
    global _NC_CACHE
    if _NC_CACHE is not None:
        return _NC_CACHE
    _apply_tile_patches()

    nc = bass.Bass("TRN2", num_devices=N_CORES, debug=False)
    xt = nc.declare_dram_parameter("xt", [BL, HC, 128, SCAT], mybir.dt.bfloat16, isOutput=False)
    xn = nc.declare_dram_parameter("xn", [BL, S, H], mybir.dt.bfloat16, isOutput=False)
    wt = nc.declare_dram_parameter("wt", [128, HC * H], mybir.dt.bfloat16, isOutput=False)
    wcx = nc.declare_dram_parameter("wcx", [128, 2 * 2 * BL * BL], mybir.dt.bfloat16, isOutput=False)
    weight = nc.declare_dram_parameter("weight", [BL, S], F32, isOutput=True)
    out = nc.declare_dram_parameter("out", [BL, H], F32, isOutput=True)

    Tanh = mybir.ActivationFunctionType.Tanh
    Exp = mybir.ActivationFunctionType.Exp
    Copy = mybir.ActivationFunctionType.Copy

    with tile.TileContext(nc) as tc:
        with (
            tc.tile_pool(name="consts", bufs=1) as consts,
            tc.tile_pool(name="xtp", bufs=4) as xtp,
            tc.tile_pool(name="xnp", bufs=8) as xnp,
            tc.tile_pool(name="thp", bufs=6) as thp,
            tc.tile_pool(name="tailp", bufs=1) as tailp,
            tc.tile_pool(name="outp", bufs=2) as outp,
            tc.tile_pool(name="mps", bufs=3, space="PSUM") as mps,
            tc.tile_pool(name="sps", bufs=2, space="PSUM") as sps,
        ):
            # constants
            wt_sb = consts.tile([128, HC, H], BF)
            nc.sync.dma_start(out=wt_sb[:], in_=wt[:])
            wcx_sb = consts.tile([128, 2, 2, BL, BL], BF)
            nc.sync.dma_start(out=wcx_sb[:], in_=wcx[:])

            # score accumulator lives in SBUF; each half's contribution is
            # produced in a small PSUM scratch and added in on DVE
            score_sb = tailp.tile([BL, S], F32)
            nc.vector.memset(score_sb[:], 0.0)

            xn_sb = {}
            for b in range(BL):
                xt_sb = [xtp.tile([128, SCAT], BF, tag="xt", name=f"xt_sb_{b}_{hc}") for hc in range(HC)]
                for half in range(2):
                    for hc in range(HC):
                        nc.sync.dma_start(
                            out=xt_sb[hc][:, half * S : (half + 1) * S],
                            in_=xt[b, hc, :, half * S : (half + 1) * S],
                        )
                for sh in range(4):  # four 1024-col halves of the concat stream
                    j = 0 if sh < 2 else 1  # w1 for text halves, w2 for aspect halves
                    col0 = (sh % 2) * 1024  # output score columns
                    th = []
                    for kc in range(2):
                        mt = mps.tile([128, 1024], F32, tag="m")
                        for hc in range(HC):
                            lhsT = wt_sb[:, hc, kc * 128 : (kc + 1) * 128]
                            for sc in range(2):
                                nc.tensor.matmul(
                                    out=mt[:, sc * 512 : (sc + 1) * 512],
                                    lhsT=lhsT,
                                    rhs=xt_sb[hc][
                                        :, sh * 1024 + sc * 512 : sh * 1024 + (sc + 1) * 512
                                    ],
                                    start=(hc == 0),
                                    stop=(hc == HC - 1),
                                )
                        tht = thp.tile([128, 1024], BF, tag="th")
                        nc.scalar.activation(out=tht[:], in_=mt[:], func=Tanh)
                        th.append(tht)
                    for sc in range(2):
                        ssc = sps.tile([BL, 512], F32, tag="ssc", name=f"ssc_{b}_{sh}_{sc}")
                        for kc in range(2):
                            nc.tensor.matmul(
                                out=ssc[:],
                                lhsT=wcx_sb[:, kc, j, b, :],
                                rhs=th[kc][:, sc * 512 : (sc + 1) * 512],
                                start=(kc == 0),
                                stop=(kc == 1),
                            )
                        c0 = col0 + sc * 512
                        nc.vector.tensor_add(
                            score_sb[:, c0 : c0 + 512],
                            score_sb[:, c0 : c0 + 512],
                            ssc[:],
                        )

                xn_sb[b] = xnp.tile([128, S // 128, H], BF, tag="xn", name=f"xn_sb_{b}")
                nc.gpsimd.dma_start(
                    out=xn_sb[b][:], in_=xn[b].rearrange("(t p) h -> p t h", p=128)
                )

            # ---- softmax tail (batched across the 8 examples) ----
            e_f32 = tailp.tile([BL, S], F32)
            z_sb = tailp.tile([BL, 1], F32)
            nc.scalar.activation(
                out=e_f32[:], in_=score_sb[:], func=Exp, accum_out=z_sb[:]
            )
            rz = tailp.tile([BL, 1], F32)
            nc.vector.reciprocal(rz[:], z_sb[:])
            nc.vector.tensor_scalar_mul(e_f32[:], e_f32[:], rz[:])
            nc.sync.dma_start(out=weight[:], in_=e_f32[:])

            # normalized weights in bf16, padded to 16 partitions for the
            # DMA xbar transpose (partition count must be a multiple of 16)
            e_bf = tailp.tile([16, S], BF)
            nc.vector.memset(e_bf[:], 0.0)
            nc.vector.tensor_copy(e_bf[0:BL, :], e_f32[:])
            et = tailp.tile([128, S // 128, 16], BF)
            nc.sync.dma_start_transpose(out=et[:], in_=e_bf[:])

            # out[b] = sum_s weight[b, s] * text[b, s, :]
            for b in range(BL):
                ops = sps.tile([1, H], F32, tag="ssc", name=f"ops_{b}")
                for c in range(S // 128):
                    nc.tensor.matmul(
                        out=ops[:],
                        lhsT=et[:, c, b : b + 1],
                        rhs=xn_sb[b][:, c, :],
                        start=(c == 0),
                        stop=(c == S // 128 - 1),
                    )
                orow = outp.tile([1, H], F32, tag="orow")
                nc.scalar.activation(out=orow[:], in_=ops[:], func=Copy)
                nc.sync.dma_start(out=out[b : b + 1, :], in_=orow[:])

    _NC_CACHE = nc
    return nc


# ---------------------------------------------------------------------------
# Host-side sharding / layout
# ---------------------------------------------------------------------------
def _prep_in_maps(text, aspect, W_text, W_combine):
    text = np.asarray(text, dtype=np.float32)
    aspect = np.asarray(aspect, dtype=np.float32)
    W_text = np.asarray(W_text, dtype=np.float32)
    W_combine = np.asarray(W_combine, dtype=np.float32)

    # [p, hc*k] layout: contiguous per partition for a clean DMA
    wt_np = np.ascontiguousarray(
        W_text.T.reshape(HC, 128, H).transpose(1, 0, 2).reshape(128, HC * H)
    ).astype(BF16)
    wcx5 = np.zeros((2, 2, BL, 128, BL), dtype=BF16)
    for kc in range(2):
        for j in range(2):
            seg = W_combine[0, j * H + kc * 128 : j * H + (kc + 1) * 128].astype(BF16)
            for b in range(BL):
                wcx5[kc, j, b, :, b] = seg
    wcx_np = np.ascontiguousarray(
        wcx5.transpose(3, 0, 1, 2, 4).reshape(128, 2 * 2 * BL * BL)
    )

    in_maps = []
    for i in range(N_CORES):
        tb = text[i * BL : (i + 1) * BL]  # [8, S, H]
        ab = aspect[i * BL : (i + 1) * BL]
        cat = np.concatenate([tb.transpose(0, 2, 1), ab.transpose(0, 2, 1)], axis=2)
        xt_np = np.ascontiguousarray(cat).astype(BF16).reshape(BL, HC, 128, SCAT)
        xn_np = tb.astype(BF16)
        in_maps.append(
            {"xt": xt_np, "xn": xn_np, "wt": wt_np, "wcx": wcx_np}
        )
    return in_maps


def _run(inputs, trace=False, tmpdir=None):
    nc = build_nc()
    in_maps = _prep_in_maps(
        inputs["text"], inputs["aspect"], inputs["W_text"], inputs["W_combine"]
    )
    res = run_bass_kernel_spmd(
        nc, in_maps, list(range(N_CORES)), trace=trace, tmpdir=tmpdir
    )
    weight = np.stack([res.results[i]["weight"] for i in range(N_CORES)]).reshape(
        B, 1, S
    )
    out = np.stack([res.results[i]["out"] for i in range(N_CORES)]).reshape(B, 1, H)
    return (weight.astype(np.float32), out.astype(np.float32)), res


def kernel(**inputs):
    outputs, _ = _run(inputs, trace=False)
    return outputs


# revision 12
# speedup vs baseline: 1.6044x; 1.3028x over previous
"""Trainium2 Bass kernel for nn_Attention_65094524338925.

Reference computation (B=64, S=2048, H=256):
    m_text   = text  @ W_text.T          [B,S,H]
    m_aspect = aspect @ W_text.T         [B,S,H]
    combine  = tanh([m_text, m_aspect])  [B,S,2H]
    score    = combine @ W_combine.T     [B,S,1]
    weight   = softmax(score, axis=1)    -> transposed to [B,1,S]
    out      = weight @ text             [B,1,H]
    returns (weight, out)

Strategy: data-parallel over batch, 8 examples per NeuronCore. The host
shards and lays out inputs so the device never needs an on-chip transpose
of the big activations:
  - xt:  concat(text.T, aspect.T) per example, bf16  [hc, 128, 4096]
         (PE contracts over the partition dim, so h lives on partitions)
  - xn:  text in natural [s, h] layout, bf16 (for the final weight @ text)
  - wt:  W_text.T chunks (stationary operand of the first matmul)
  - wcx: W_combine halves as one-hot-column [128, 8] stationary tiles
         (matmul PSUM output must start at partition 0/32/64, so each
         example's score row is produced by an M=8 matmul whose lhsT has
         only column b nonzero)

Per example: PE computes m.T [k, s] in PSUM, ACT applies tanh (psum->sbuf,
bf16), PE dots tanh rows with W_combine into a shared [8, 2048] score
accumulator. Tail: one batched exp with fused row-sum (softmax without max
subtraction -- scores are O(0.1) so exp cannot overflow), normalize on DVE,
DMA-transpose the weights to [s, b] layout, and PE reduces weight @ text.
"""

import numpy as np
import ml_dtypes

import concourse.bass as bass
import concourse.mybir as mybir
import concourse.tile as tile
from concourse.bass_utils import run_bass_kernel_spmd

BF16 = ml_dtypes.bfloat16
FP8 = mybir.dt.np(mybir.dt.float8e4)
F32 = mybir.dt.float32
BF = mybir.dt.bfloat16
F8 = mybir.dt.float8e4
W_SCALE = 1024.0

N_CORES = 8
B, S, H = 64, 2048, 256
BL = B // N_CORES  # 8 examples per core
HC = H // 128  # 2 h-chunks
SCAT = 2 * S  # concat(text, aspect) along s: 4096


# ---------------------------------------------------------------------------
# Workaround: this walrus build accepts at most one sync wait per TPB
# instruction (64B instruction formats carry a single wait slot). Tile's
# scheduler attaches one wait per outstanding proc, so (1) spill excess
# waits onto preceding same-engine NOPs and (2) replace the exit drain's
# multi-proc wait list with per-proc sync NOPs.
# ---------------------------------------------------------------------------
_PATCHED = False


def _apply_tile_patches():
    global _PATCHED
    if _PATCHED:
        return
    _PATCHED = True
    from concourse.vector_clock import ScopedClock, VectorClock

    max_waits = 1
    ctr = [0]
    orig_lower = tile.TileContext._lower_ordered_insts

    def split_waits(ordered):
        for bb_name, insts in ordered.items():
            new = []
            for inst in insts:
                si = inst.sync_info
                if si is not None and si.on_wait and len(si.on_wait) > max_waits:
                    waits = list(si.on_wait)
                    spill, keep = waits[:-max_waits], waits[-max_waits:]
                    for w in spill:
                        ctr[0] += 1
                        new.append(
                            mybir.InstNoOp(
                                name=f"waitspill_{ctr[0]}",
                                engine=inst.engine,
                                sync_info=mybir.SyncInfo(on_wait=[w], on_update=[]),
                                bass_nofuse=True,
                            )
                        )
                    inst.sync_info = mybir.SyncInfo(
                        on_wait=keep, on_update=list(si.on_update or [])
                    )
                new.append(inst)
            ordered[bb_name] = new

    def patched_lower(self, ordered):
        split_waits(ordered)
        return orig_lower(self, ordered)

    def patched_drain_and_barrier(self, tick_clock, wait_clock):
        gc = tick_clock.global_clock
        n = len(gc)
        for p in range(n):
            t = gc[p]
            if t <= 0:
                continue
            vec = [0] * n
            vec[p] = t
            nop = self.nc.sync.nop()
            wait_clock.add_sem_waits(nop.ins, ScopedClock({None: VectorClock(vec)}))
        self.nc.sync.drain()
        self.nc.all_engine_barrier()
        assert self.sems is not None
        popped = self.nc._tile_sem_poison_stack.pop()
        assert popped is self._sem_poison
        self.nc.clear_and_free_semaphores(list(self.sems.allocated().values()))
        self.nc.all_engine_barrier()

    tile.TileContext._lower_ordered_insts = patched_lower
    tile.TileContext._drain_and_barrier = patched_drain_and_barrier


# ---------------------------------------------------------------------------
# Kernel build
# ---------------------------------------------------------------------------
_NC_CACHE = None


def build_nc():
    global _NC_CACHE
    if _NC_CACHE is not None:
        return _NC_CACHE
    _apply_tile_patches()

    nc = bass.Bass("TRN2", num_devices=N_CORES, debug=False)
    xt = nc.declare_dram_parameter("xt", [BL, 128, HC, SCAT], F8, isOutput=False)
    xn = nc.declare_dram_parameter("xn", [BL, S, H], mybir.dt.bfloat16, isOutput=False)
    wt = nc.declare_dram_parameter("wt", [128, HC * H], F8, isOutput=False)
    wcx = nc.declare_dram_parameter("wcx", [128, 2 * 2 * BL * BL], mybir.dt.bfloat16, isOutput=False)
    weight = nc.declare_dram_parameter("weight", [BL, S], F32, isOutput=True)
    out = nc.declare_dram_parameter("out", [BL, H], F32, isOutput=True)

    Tanh = mybir.ActivationFunctionType.Tanh
    Exp = mybir.ActivationFunctionType.Exp
    Copy = mybir.ActivationFunctionType.Copy

    with tile.TileContext(nc) as tc:
        with (
            tc.tile_pool(name="consts", bufs=1) as consts,
            tc.tile_pool(name="xtp", bufs=4) as xtp,
            tc.tile_pool(name="xnp", bufs=8) as xnp,
            tc.tile_pool(name="thp", bufs=6) as thp,
            tc.tile_pool(name="tailp", bufs=1) as tailp,
            tc.tile_pool(name="outp", bufs=2) as outp,
            tc.tile_pool(name="mps", bufs=3, space="PSUM") as mps,
            tc.tile_pool(name="sps", bufs=2, space="PSUM") as sps,
        ):
            # constants
            wt_sb = consts.tile([128, HC, H], F8)
            nc.sync.dma_start(out=wt_sb[:], in_=wt[:])
            wcx_sb = consts.tile([128, 2, 2, BL, BL], BF)
            nc.sync.dma_start(out=wcx_sb[:], in_=wcx[:])

            # score accumulator lives in SBUF; each half's contribution is
            # produced in a small PSUM scratch and added in on DVE
            score_sb = tailp.tile([BL, S], F32)
            nc.vector.memset(score_sb[:], 0.0)

            xn_sb = {}
            for b in range(BL):
                xt_sb = xtp.tile([128, HC, SCAT], F8, tag="xt", name=f"xt_sb_{b}")
                for half in range(2):
                    nc.sync.dma_start(
                        out=xt_sb[:, :, half * S : (half + 1) * S],
                        in_=xt[b, :, :, half * S : (half + 1) * S],
                    )
                for sh in range(4):  # four 1024-col halves of the concat stream
                    j = 0 if sh < 2 else 1  # w1 for text halves, w2 for aspect halves
                    col0 = (sh % 2) * 1024  # output score columns
                    th = []
                    for kc in range(2):
                        mt = mps.tile([128, 1024], F32, tag="m")
                        lhsT = wt_sb[:, :, kc * 128 : (kc + 1) * 128]
                        for sc in range(2):
                            nc.tensor.matmul(
                                out=mt[:, sc * 512 : (sc + 1) * 512],
                                lhsT=lhsT,
                                rhs=xt_sb[
                                    :, :, sh * 1024 + sc * 512 : sh * 1024 + (sc + 1) * 512
                                ],
                                start=True,
                                stop=True,
                                perf_mode=mybir.MatmulPerfMode.DoubleRow,
                            )
                        tht = thp.tile([128, 1024], BF, tag="th")
                        nc.scalar.activation(
                            out=tht[:], in_=mt[:], func=Tanh, scale=1.0 / W_SCALE
                        )
                        th.append(tht)
                    for sc in range(2):
                        ssc = sps.tile([BL, 512], F32, tag="ssc", name=f"ssc_{b}_{sh}_{sc}")
                        for kc in range(2):
                            nc.tensor.matmul(
                                out=ssc[:],
                                lhsT=wcx_sb[:, kc, j, b, :],
                                rhs=th[kc][:, sc * 512 : (sc + 1) * 512],
                                start=(kc == 0),
                                stop=(kc == 1),
                            )
                        c0 = col0 + sc * 512
                        nc.vector.tensor_add(
                            score_sb[:, c0 : c0 + 512],
                            score_sb[:, c0 : c0 + 512],
                            ssc[:],
                        )

                xn_sb[b] = xnp.tile([128, S // 128, H], BF, tag="xn", name=f"xn_sb_{b}")
                nc.gpsimd.dma_start(
                    out=xn_sb[b][:], in_=xn[b].rearrange("(t p) h -> p t h", p=128)
                )

            # ---- softmax tail (batched across the 8 examples) ----
            e_f32 = tailp.tile([BL, S], F32)
            z_sb = tailp.tile([BL, 1], F32)
            nc.scalar.activation(
                out=e_f32[:], in_=score_sb[:], func=Exp, accum_out=z_sb[:]
            )
            rz = tailp.tile([BL, 1], F32)
            nc.vector.reciprocal(rz[:], z_sb[:])
            nc.vector.tensor_scalar_mul(e_f32[:], e_f32[:], rz[:])
            nc.sync.dma_start(out=weight[:], in_=e_f32[:])

            # normalized weights in bf16, padded to 16 partitions for the
            # DMA xbar transpose (partition count must be a multiple of 16)
            e_bf = tailp.tile([16, S], BF)
            nc.vector.memset(e_bf[:], 0.0)
            nc.vector.tensor_copy(e_bf[0:BL, :], e_f32[:])
            et = tailp.tile([128, S // 128, 16], BF)
            nc.sync.dma_start_transpose(out=et[:], in_=e_bf[:])

            # out[b] = sum_s weight[b, s] * text[b, s, :]
            for b in range(BL):
                ops = sps.tile([1, H], F32, tag="ssc", name=f"ops_{b}")
                for c in range(S // 128):
                    nc.tensor.matmul(
                        out=ops[:],
                        lhsT=et[:, c, b : b + 1],
                        rhs=xn_sb[b][:, c, :],
                        start=(c == 0),
                        stop=(c == S // 128 - 1),
                    )
                orow = outp.tile([1, H], F32, tag="orow")
                nc.scalar.activation(out=orow[:], in_=ops[:], func=Copy)
                nc.sync.dma_start(out=out[b : b + 1, :], in_=orow[:])

    _NC_CACHE = nc
    return nc


# ---------------------------------------------------------------------------
# Host-side sharding / layout
# ---------------------------------------------------------------------------
def _prep_in_maps(text, aspect, W_text, W_combine):
    text = np.asarray(text, dtype=np.float32)
    aspect = np.asarray(aspect, dtype=np.float32)
    W_text = np.asarray(W_text, dtype=np.float32)
    W_combine = np.asarray(W_combine, dtype=np.float32)

    # [ki, ko*k] layout (DoubleRow packing), scaled into fp8 normal range
    wt_np = np.ascontiguousarray(
        (W_SCALE * W_text.T).reshape(HC, 128, H).transpose(1, 0, 2).reshape(128, HC * H)
    ).astype(FP8)
    wcx5 = np.zeros((2, 2, BL, 128, BL), dtype=BF16)
    for kc in range(2):
        for j in range(2):
            seg = W_combine[0, j * H + kc * 128 : j * H + (kc + 1) * 128].astype(BF16)
            for b in range(BL):
                wcx5[kc, j, b, :, b] = seg
    wcx_np = np.ascontiguousarray(
        wcx5.transpose(3, 0, 1, 2, 4).reshape(128, 2 * 2 * BL * BL)
    )

    in_maps = []
    for i in range(N_CORES):
        tb = text[i * BL : (i + 1) * BL]  # [8, S, H]
        ab = aspect[i * BL : (i + 1) * BL]
        cat = np.concatenate([tb.transpose(0, 2, 1), ab.transpose(0, 2, 1)], axis=2)
        xt_np = np.ascontiguousarray(
            cat.reshape(BL, HC, 128, SCAT).transpose(0, 2, 1, 3)
        ).astype(FP8)
        xn_np = tb.astype(BF16)
        in_maps.append(
            {"xt": xt_np, "xn": xn_np, "wt": wt_np, "wcx": wcx_np}
        )
    return in_maps


def _run(inputs, trace=False, tmpdir=None):
    nc = build_nc()
    in_maps = _prep_in_maps(
        inputs["text"], inputs["aspect"], inputs["W_text"], inputs["W_combine"]
    )
    res = run_bass_kernel_spmd(
        nc, in_maps, list(range(N_CORES)), trace=trace, tmpdir=tmpdir
    )
    weight = np.stack([res.results[i]["weight"] for i in range(N_CORES)]).reshape(
        B, 1, S
    )
    out = np.stack([res.results[i]["out"] for i in range(N_CORES)]).reshape(B, 1, H)
    return (weight.astype(np.float32), out.astype(np.float32)), res


def kernel(**inputs):
    outputs, _ = _run(inputs, trace=False)
    return outputs


# revision 16
# speedup vs baseline: 1.7580x; 1.0957x over previous
"""Trainium2 Bass kernel for nn_Attention_65094524338925.

Reference computation (B=64, S=2048, H=256):
    m_text   = text  @ W_text.T          [B,S,H]
    m_aspect = aspect @ W_text.T         [B,S,H]
    combine  = tanh([m_text, m_aspect])  [B,S,2H]
    score    = combine @ W_combine.T     [B,S,1]
    weight   = softmax(score, axis=1)    -> transposed to [B,1,S]
    out      = weight @ text             [B,1,H]
    returns (weight, out)

Strategy: data-parallel over batch, 8 examples per NeuronCore. The host
shards and lays out inputs so the device never needs an on-chip transpose
of the big activations:
  - xt:  concat(text.T, aspect.T) per example, bf16  [hc, 128, 4096]
         (PE contracts over the partition dim, so h lives on partitions)
  - xn:  text in natural [s, h] layout, bf16 (for the final weight @ text)
  - wt:  W_text.T chunks (stationary operand of the first matmul)
  - wcx: W_combine halves as one-hot-column [128, 8] stationary tiles
         (matmul PSUM output must start at partition 0/32/64, so each
         example's score row is produced by an M=8 matmul whose lhsT has
         only column b nonzero)

Per example: PE computes m.T [k, s] in PSUM, ACT applies tanh (psum->sbuf,
bf16), PE dots tanh rows with W_combine into a shared [8, 2048] score
accumulator. Tail: one batched exp with fused row-sum (softmax without max
subtraction -- scores are O(0.1) so exp cannot overflow), normalize on DVE,
DMA-transpose the weights to [s, b] layout, and PE reduces weight @ text.
"""

import numpy as np
import ml_dtypes

import concourse.bass as bass
import concourse.mybir as mybir
import concourse.tile as tile
from concourse.bass_utils import run_bass_kernel_spmd

BF16 = ml_dtypes.bfloat16
FP8 = mybir.dt.np(mybir.dt.float8e4)
F32 = mybir.dt.float32
BF = mybir.dt.bfloat16
F8 = mybir.dt.float8e4
W_SCALE = 1024.0

N_CORES = 8
B, S, H = 64, 2048, 256
BL = B // N_CORES  # 8 examples per core
HC = H // 128  # 2 h-chunks
SCAT = 2 * S  # concat(text, aspect) along s: 4096


# ---------------------------------------------------------------------------
# Workaround: this walrus build accepts at most one sync wait per TPB
# instruction (64B instruction formats carry a single wait slot). Tile's
# scheduler attaches one wait per outstanding proc, so (1) spill excess
# waits onto preceding same-engine NOPs and (2) replace the exit drain's
# multi-proc wait list with per-proc sync NOPs.
# ---------------------------------------------------------------------------
_PATCHED = False


def _apply_tile_patches():
    global _PATCHED
    if _PATCHED:
        return
    _PATCHED = True
    from concourse.vector_clock import ScopedClock, VectorClock

    max_waits = 1
    ctr = [0]
    orig_lower = tile.TileContext._lower_ordered_insts

    def split_waits(ordered):
        for bb_name, insts in ordered.items():
            new = []
            for inst in insts:
                si = inst.sync_info
                if si is not None and si.on_wait and len(si.on_wait) > max_waits:
                    waits = list(si.on_wait)
                    spill, keep = waits[:-max_waits], waits[-max_waits:]
                    for w in spill:
                        ctr[0] += 1
                        new.append(
                            mybir.InstNoOp(
                                name=f"waitspill_{ctr[0]}",
                                engine=inst.engine,
                                sync_info=mybir.SyncInfo(on_wait=[w], on_update=[]),
                                bass_nofuse=True,
                            )
                        )
                    inst.sync_info = mybir.SyncInfo(
                        on_wait=keep, on_update=list(si.on_update or [])
                    )
                new.append(inst)
            ordered[bb_name] = new

    def patched_lower(self, ordered):
        split_waits(ordered)
        return orig_lower(self, ordered)

    def patched_drain_and_barrier(self, tick_clock, wait_clock):
        gc = tick_clock.global_clock
        n = len(gc)
        for p in range(n):
            t = gc[p]
            if t <= 0:
                continue
            vec = [0] * n
            vec[p] = t
            nop = self.nc.sync.nop()
            wait_clock.add_sem_waits(nop.ins, ScopedClock({None: VectorClock(vec)}))
        self.nc.sync.drain()
        self.nc.all_engine_barrier()
        assert self.sems is not None
        popped = self.nc._tile_sem_poison_stack.pop()
        assert popped is self._sem_poison
        self.nc.clear_and_free_semaphores(list(self.sems.allocated().values()))
        self.nc.all_engine_barrier()

    tile.TileContext._lower_ordered_insts = patched_lower
    tile.TileContext._drain_and_barrier = patched_drain_and_barrier


# ---------------------------------------------------------------------------
# Kernel build
# ---------------------------------------------------------------------------
_NC_CACHE = None


def build_nc():
    global _NC_CACHE
    if _NC_CACHE is not None:
        return _NC_CACHE
    _apply_tile_patches()

    nc = bass.Bass("TRN2", num_devices=N_CORES, debug=False)
    xt = nc.declare_dram_parameter("xt", [BL, 128, HC, SCAT], F8, isOutput=False)
    xn = nc.declare_dram_parameter("xn", [BL, S, H], mybir.dt.bfloat16, isOutput=False)
    wt = nc.declare_dram_parameter("wt", [128, HC * H], F8, isOutput=False)
    wcx = nc.declare_dram_parameter("wcx", [128, 2 * 2 * BL * BL], mybir.dt.bfloat16, isOutput=False)
    eye8 = nc.declare_dram_parameter("eye8", [BL, BL], F32, isOutput=False)
    weight = nc.declare_dram_parameter("weight", [BL, S], F32, isOutput=True)
    out = nc.declare_dram_parameter("out", [BL, H], F32, isOutput=True)

    Tanh = mybir.ActivationFunctionType.Tanh
    Exp = mybir.ActivationFunctionType.Exp
    Copy = mybir.ActivationFunctionType.Copy

    with tile.TileContext(nc) as tc:
        with (
            tc.tile_pool(name="consts", bufs=1) as consts,
            tc.tile_pool(name="xtp", bufs=4) as xtp,
            tc.tile_pool(name="xnp", bufs=8) as xnp,
            tc.tile_pool(name="thp", bufs=6) as thp,
            tc.tile_pool(name="tailp", bufs=1) as tailp,
            tc.tile_pool(name="outp", bufs=2) as outp,
            tc.tile_pool(name="mps", bufs=3, space="PSUM") as mps,
            tc.tile_pool(name="sps", bufs=2, space="PSUM") as sps,
        ):
            # constants
            wt_sb = consts.tile([128, HC, H], F8)
            nc.sync.dma_start(out=wt_sb[:], in_=wt[:])
            wcx_sb = consts.tile([128, 2, 2, BL, BL], BF)
            nc.sync.dma_start(out=wcx_sb[:], in_=wcx[:])

            # score accumulator lives in SBUF; each half's contribution is
            # produced in a small PSUM scratch and added in on DVE
            score_sb = tailp.tile([BL, S], F32)
            nc.vector.memset(score_sb[:], 0.0)

            xn_sb = {}
            for b in range(BL):
                xt_sb = xtp.tile([128, HC, SCAT], F8, tag="xt", name=f"xt_sb_{b}")
                for half in range(2):
                    nc.sync.dma_start(
                        out=xt_sb[:, :, half * S : (half + 1) * S],
                        in_=xt[b, :, :, half * S : (half + 1) * S],
                    )
                for sh in range(4):  # four 1024-col halves of the concat stream
                    j = 0 if sh < 2 else 1  # w1 for text halves, w2 for aspect halves
                    col0 = (sh % 2) * 1024  # output score columns
                    th = []
                    for kc in range(2):
                        mt = mps.tile([128, 1024], F32, tag="m")
                        lhsT = wt_sb[:, :, kc * 128 : (kc + 1) * 128]
                        for sc in range(2):
                            nc.tensor.matmul(
                                out=mt[:, sc * 512 : (sc + 1) * 512],
                                lhsT=lhsT,
                                rhs=xt_sb[
                                    :, :, sh * 1024 + sc * 512 : sh * 1024 + (sc + 1) * 512
                                ],
                                start=True,
                                stop=True,
                                perf_mode=mybir.MatmulPerfMode.DoubleRow,
                            )
                        tht = thp.tile([128, 1024], BF, tag="th")
                        nc.scalar.activation(
                            out=tht[:], in_=mt[:], func=Tanh, scale=1.0 / W_SCALE
                        )
                        th.append(tht)
                    for sc in range(2):
                        ssc = sps.tile([BL, 512], F32, tag="ssc", name=f"ssc_{b}_{sh}_{sc}")
                        for kc in range(2):
                            nc.tensor.matmul(
                                out=ssc[:],
                                lhsT=wcx_sb[:, kc, j, b, :],
                                rhs=th[kc][:, sc * 512 : (sc + 1) * 512],
                                start=(kc == 0),
                                stop=(kc == 1),
                            )
                        c0 = col0 + sc * 512
                        nc.vector.tensor_add(
                            score_sb[:, c0 : c0 + 512],
                            score_sb[:, c0 : c0 + 512],
                            ssc[:],
                        )

                xn_sb[b] = xnp.tile([128, S // 128, H], BF, tag="xn", name=f"xn_sb_{b}")
                # tiny write that depends on this example's last score add: the
                # full-tile DMA below must wait for it (WAW), so xn loads
                # trickle in behind compute instead of racing the xt loads
                nc.vector.tensor_copy(
                    xn_sb[b][0:1, 0:1, 0:1], score_sb[0:1, b : b + 1]
                )
                nc.gpsimd.dma_start(
                    out=xn_sb[b][:], in_=xn[b].rearrange("(t p) h -> p t h", p=128)
                )

            # ---- softmax tail (batched across the 8 examples) ----
            # exp writes UNNORMALIZED bf16 weights straight into the padded
            # transpose source; row sums come out of the same instruction
            e_bf = tailp.tile([16, S], BF)
            nc.vector.memset(e_bf[:], 0.0)
            z_sb = tailp.tile([BL, 1], F32)
            nc.scalar.activation(
                out=e_bf[0:BL, :], in_=score_sb[:], func=Exp, accum_out=z_sb[:]
            )
            et = tailp.tile([128, S // 128, 16], BF)
            nc.sync.dma_start_transpose(out=et[:], in_=e_bf[:])

            # off the critical path: f32 normalized weight output
            rz = tailp.tile([BL, 1], F32)
            nc.vector.reciprocal(rz[:], z_sb[:])
            e_f32 = tailp.tile([BL, S], F32)
            nc.vector.tensor_scalar_mul(e_f32[:], e_bf[0:BL, :], rz[:])
            nc.sync.dma_start(out=weight[:], in_=e_f32[:])

            # 1/Z as a row vector on partition 0 (for the final out scaling):
            # transpose z via a tiny identity matmul (z.T @ I)
            eye_sb = consts.tile([BL, BL], F32)
            nc.sync.dma_start(out=eye_sb[:], in_=eye8[:])
            zrow_ps = sps.tile([1, BL], F32, tag="ssc", name="zrow_ps")
            nc.tensor.matmul(
                out=zrow_ps[:], lhsT=z_sb[:], rhs=eye_sb[:], start=True, stop=True
            )
            zrow_inv = tailp.tile([1, BL], F32)
            nc.vector.reciprocal(zrow_inv[:], zrow_ps[:])

            # out[b] = sum_s weight[b, s] * text[b, s, :]
            for b in range(BL):
                ops = sps.tile([1, H], F32, tag="ssc", name=f"ops_{b}")
                for c in range(S // 128):
                    nc.tensor.matmul(
                        out=ops[:],
                        lhsT=et[:, c, b : b + 1],
                        rhs=xn_sb[b][:, c, :],
                        start=(c == 0),
                        stop=(c == S // 128 - 1),
                    )
                orow = outp.tile([1, H], F32, tag="orow")
                nc.scalar.activation(
                    out=orow[:], in_=ops[:], func=Copy,
                    scale=zrow_inv[0:1, b : b + 1],
                )
                nc.sync.dma_start(out=out[b : b + 1, :], in_=orow[:])

    _NC_CACHE = nc
    return nc


# ---------------------------------------------------------------------------
# Host-side sharding / layout
# ---------------------------------------------------------------------------
def _prep_in_maps(text, aspect, W_text, W_combine):
    text = np.asarray(text, dtype=np.float32)
    aspect = np.asarray(aspect, dtype=np.float32)
    W_text = np.asarray(W_text, dtype=np.float32)
    W_combine = np.asarray(W_combine, dtype=np.float32)

    # [ki, ko*k] layout (DoubleRow packing), scaled into fp8 normal range
    wt_np = np.ascontiguousarray(
        (W_SCALE * W_text.T).reshape(HC, 128, H).transpose(1, 0, 2).reshape(128, HC * H)
    ).astype(FP8)
    wcx5 = np.zeros((2, 2, BL, 128, BL), dtype=BF16)
    for kc in range(2):
        for j in range(2):
            seg = W_combine[0, j * H + kc * 128 : j * H + (kc + 1) * 128].astype(BF16)
            for b in range(BL):
                wcx5[kc, j, b, :, b] = seg
    wcx_np = np.ascontiguousarray(
        wcx5.transpose(3, 0, 1, 2, 4).reshape(128, 2 * 2 * BL * BL)
    )

    in_maps = []
    for i in range(N_CORES):
        tb = text[i * BL : (i + 1) * BL]  # [8, S, H]
        ab = aspect[i * BL : (i + 1) * BL]
        cat = np.concatenate([tb.transpose(0, 2, 1), ab.transpose(0, 2, 1)], axis=2)
        xt_np = np.ascontiguousarray(
            cat.reshape(BL, HC, 128, SCAT).transpose(0, 2, 1, 3)
        ).astype(FP8)
        xn_np = tb.astype(BF16)
        in_maps.append(
            {"xt": xt_np, "xn": xn_np, "wt": wt_np, "wcx": wcx_np,
             "eye8": np.eye(BL, dtype=np.float32)}
        )
    return in_maps


def _run(inputs, trace=False, tmpdir=None):
    nc = build_nc()
    in_maps = _prep_in_maps(
        inputs["text"], inputs["aspect"], inputs["W_text"], inputs["W_combine"]
    )
    res = run_bass_kernel_spmd(
        nc, in_maps, list(range(N_CORES)), trace=trace, tmpdir=tmpdir
    )
    weight = np.stack([res.results[i]["weight"] for i in range(N_CORES)]).reshape(
        B, 1, S
    )
    out = np.stack([res.results[i]["out"] for i in range(N_CORES)]).reshape(B, 1, H)
    return (weight.astype(np.float32), out.astype(np.float32)), res


def kernel(**inputs):
    outputs, _ = _run(inputs, trace=False)
    return outputs
